# revision 13
# baseline (speedup 1.0000x reference)
"""Cross-modal triplet loss (hardest pos/neg mining) on 8 TRN2 NeuronCores.

Strategy
--------
Rows of the four 4096x4096 distance matrices are sharded across 8 cores
(512 rows each).  On the host we sort rows/columns by target id so the
same-identity mask becomes one contiguous column range per row; each core's
column order is additionally rotated so its own diagonal block lands near
column 64, which keeps every class range inside a static 320-wide window of
the first PSUM half.  On device, per (128-row block, matrix):

  PE   : E = sq_b[j] - 2*a_i.b_j accumulated in PSUM via a K=1 "bias" matmul
         (ones x sq_b) followed by the K=128 data matmul (float32r).
  DVE  : TENSOR_MASK_REDUCE gives the hardest positive (masked max over the
         class range, static diag window) and hardest negative (max of -E
         over the wrap-inverted class range, negated at the end) without
         materializing any mask.
  ACT  : relu(x + sq_a) and sqrt(x + 1e-12) on the [128, 8] reduced columns.
  DVE  : margin terms relu(ap - an + 0.3) and (ap < an) counts, accumulated
         across blocks into a [128, 2] partial-sum tile.

The host sums the 8 x [128, 2] partials and divides by 6*n.  Sorting is a
permutation of rows, and loss/prec are means over rows, so no un-permutation
is needed.
"""

import sys

import numpy as np

for _p in ("/opt/trn_rl_repo", "/root/.axon_site/_ro/trn_rl_repo"):
    if _p not in sys.path:
        sys.path.append(_p)

import concourse.bacc as bacc
import concourse.mybir as mybir
import concourse.tile as tile
from concourse.bass_utils import run_bass_kernel_spmd
from concourse.dve_ops import TENSOR_MASK_REDUCE

N = 4096
D = 128
NCORES = 8
RPC = N // NCORES          # rows per core
NBLK = RPC // 128          # row blocks of 128 per core
HALF = 2048                # columns per PSUM tile (4 banks)
WIN = 320                  # static window containing every class range of a block
ROT_MARGIN = 64            # column rotation margin (max supported class size)
MARGIN = 0.3
NEG_INF = -3.4e38

F32 = mybir.dt.float32
F32R = mybir.dt.float32r
OP = mybir.AluOpType
ACTF = mybir.ActivationFunctionType


def _build_program(mm_dtype=F32R):
    nc = bacc.Bacc(
        "TRN2",
        target_bir_lowering=False,
        debug=False,
        num_devices=NCORES,
    )

    rhs1_d = nc.dram_tensor("rhs1", [D, N], mm_dtype, kind="ExternalInput")
    rhs2_d = nc.dram_tensor("rhs2", [D, N], mm_dtype, kind="ExternalInput")
    sqb1_d = nc.dram_tensor("sqb1", [1, N], mm_dtype, kind="ExternalInput")
    sqb2_d = nc.dram_tensor("sqb2", [1, N], mm_dtype, kind="ExternalInput")
    lhs1_d = nc.dram_tensor("lhs1", [D, RPC], mm_dtype, kind="ExternalInput")
    lhs2_d = nc.dram_tensor("lhs2", [D, RPC], mm_dtype, kind="ExternalInput")
    pack_d = nc.dram_tensor("pack", [128, 24], F32, kind="ExternalInput")
    ones_d = nc.dram_tensor("ones", [1, 128], mm_dtype, kind="ExternalInput")
    out_d = nc.dram_tensor("out", [128, 2], F32, kind="ExternalOutput")

    with tile.TileContext(nc) as tc:
        with (
            tc.tile_pool(name="consts", bufs=1) as cpool,
            tc.tile_pool(name="work", bufs=2) as wpool,
            tc.tile_pool(name="ps", bufs=2, space="PSUM") as pspool,
        ):
            rhs1 = cpool.tile([D, N], mm_dtype, tag="rhs1")
            nc.sync.dma_start(out=rhs1[:, :], in_=rhs1_d[:, :])
            rhs2 = cpool.tile([D, N], mm_dtype, tag="rhs2")
            nc.sync.dma_start(out=rhs2[:, :], in_=rhs2_d[:, :])
            sqb1 = cpool.tile([1, N], mm_dtype, tag="sqb1")
            nc.sync.dma_start(out=sqb1[:, :], in_=sqb1_d[:, :])
            sqb2 = cpool.tile([1, N], mm_dtype, tag="sqb2")
            nc.sync.dma_start(out=sqb2[:, :], in_=sqb2_d[:, :])
            lhs1 = cpool.tile([D, RPC], mm_dtype, tag="lhs1")
            nc.sync.dma_start(out=lhs1[:, :], in_=lhs1_d[:, :])
            lhs2 = cpool.tile([D, RPC], mm_dtype, tag="lhs2")
            nc.sync.dma_start(out=lhs2[:, :], in_=lhs2_d[:, :])
            pack = cpool.tile([128, 24], F32, tag="pack")
            nc.sync.dma_start(out=pack[:, :], in_=pack_d[:, :])

            ones1 = cpool.tile([1, 128], mm_dtype, tag="ones1")
            nc.sync.dma_start(out=ones1[:, :], in_=ones_d[:, :])
            end_half = cpool.tile([128, 1], F32, tag="end_half")
            nc.gpsimd.memset(end_half[:, :], float(HALF))
            zeros6 = cpool.tile([128, 6], F32, tag="zeros6")
            nc.gpsimd.memset(zeros6[:, :], 0.0)
            eps1 = cpool.tile([128, 1], F32, tag="eps1")
            nc.gpsimd.memset(eps1[:, :], 1e-12)
            accum = cpool.tile([128, 2], F32, tag="accum")
            nc.vector.memset(accum[:, :], 0.0)

            # (lhsT, rhs, sqb, sqa pack-column base) per distance matrix:
            # r=(m1,m1), t=(m2,m2), rt=(m1,m2), tr=(m2,m1)
            mats = [
                (lhs1, rhs1, sqb1, 16),
                (lhs2, rhs2, sqb2, 20),
                (lhs1, rhs2, sqb2, 16),
                (lhs2, rhs1, sqb1, 20),
            ]

            for b in range(NBLK):
                acc = wpool.tile([128, 8], F32, tag="acc")
                for mi, (lhsT, rhs, sqb, _) in enumerate(mats):
                    negtmp = wpool.tile([128, 1], F32, tag="negtmp")
                    for h in range(2):
                        ps = pspool.tile([128, HALF], F32, tag="ps")
                        for k in range(HALF // 512):
                            c0 = h * HALF + k * 512
                            sl = slice(k * 512, (k + 1) * 512)
                            nc.tensor.matmul(
                                out=ps[:, sl],
                                lhsT=ones1[:, :],
                                rhs=sqb[:, c0 : c0 + 512],
                                start=True,
                                stop=False,
                            )
                            nc.tensor.matmul(
                                out=ps[:, sl],
                                lhsT=lhsT[:, b * 128 : (b + 1) * 128],
                                rhs=rhs[:, c0 : c0 + 512],
                                start=False,
                                stop=True,
                            )
                        scratch = wpool.tile([128, HALF], F32, tag="scratch")
                        if h == 0:
                            # PSUM holds F = 2ab - sq_b = -E.  Negate the
                            # static diag window on ACT to recover E for the
                            # positive side (masked-out -FLT_MAX must lose a
                            # max-reduce, so data signs are arranged so both
                            # reduces are max-reduces over masked-in sets).
                            winE = wpool.tile([128, WIN], F32, tag="winE")
                            nc.scalar.activation(
                                out=winE[:, :],
                                in_=ps[:, b * 128 : b * 128 + WIN],
                                func=ACTF.Copy,
                                scale=-1.0,
                            )
                            # hardest positive: masked max of E over the class
                            nc.vector._custom_dve(
                                TENSOR_MASK_REDUCE,
                                out=scratch[:, 0:WIN],
                                in0=winE[:, :],
                                s0=pack[:, 0 + b : 1 + b],
                                in1=pack[:, 4 + b : 5 + b],
                                s1=NEG_INF,
                                imm2=1.0,
                                accum_out=acc[:, mi : mi + 1],
                            )
                            # hardest negative, half 0: max of F over the
                            # wrap-inverted class range (= complement)
                            nc.vector._custom_dve(
                                TENSOR_MASK_REDUCE,
                                out=scratch[:, :],
                                in0=ps[:, :],
                                s0=pack[:, 8 + b : 9 + b],
                                in1=pack[:, 12 + b : 13 + b],
                                s1=NEG_INF,
                                imm2=1.0,
                                accum_out=negtmp[:, :],
                            )
                        else:
                            # hardest negative, half 1: no class columns here
                            # (all-ones mask); chain.  acc[:, 4+mi] holds
                            # max(F) over the complement = -min(E); the tail
                            # Relu negates it via scale=-1.
                            nc.vector._custom_dve(
                                TENSOR_MASK_REDUCE,
                                out=scratch[:, :],
                                in0=ps[:, :],
                                s0=0.0,
                                in1=end_half[:, :],
                                s1=negtmp[:, :],
                                imm2=1.0,
                                accum_out=acc[:, 4 + mi : 5 + mi],
                            )

                # tail: ap/an for the 4 matrices live in acc cols 0-3 / 4-7
                sq = wpool.tile([128, 8], F32, tag="sq")
                # A-side of matrices [m1, m2, m1, m2] -> even cols use sq_a of
                # m1, odd cols sq_a of m2 (for both pos 0-3 and neg 4-7).
                # Neg cols hold max(F) = -min(E): scale=-1 folds the negation.
                for cols, scale in ((slice(0, 4), 1.0), (slice(4, 8), -1.0)):
                    a3 = acc[:, cols].rearrange("p (f two) -> p f two", two=2)
                    s3 = sq[:, cols].rearrange("p (f two) -> p f two", two=2)
                    nc.scalar.activation(
                        out=s3[:, :, 0:1],
                        in_=a3[:, :, 0:1],
                        func=ACTF.Relu,
                        scale=scale,
                        bias=pack[:, 16 + b : 17 + b],
                    )
                    nc.scalar.activation(
                        out=s3[:, :, 1:2],
                        in_=a3[:, :, 1:2],
                        func=ACTF.Relu,
                        scale=scale,
                        bias=pack[:, 20 + b : 21 + b],
                    )
                nc.scalar.activation(
                    out=sq[:, :], in_=sq[:, :], func=ACTF.Sqrt, bias=eps1[:, :]
                )

                # margin ranking terms over the 6 (ap, an) list pairs:
                # (0,4) (1,5) (2,6) (3,7) (2,4) (3,5)
                d6 = wpool.tile([128, 6], F32, tag="d6")
                nc.vector.scalar_tensor_tensor(
                    out=d6[:, 0:4],
                    in0=sq[:, 0:4],
                    scalar=MARGIN,
                    in1=sq[:, 4:8],
                    op0=OP.add,
                    op1=OP.subtract,
                )
                nc.vector.scalar_tensor_tensor(
                    out=d6[:, 4:6],
                    in0=sq[:, 2:4],
                    scalar=MARGIN,
                    in1=sq[:, 4:6],
                    op0=OP.add,
                    op1=OP.subtract,
                )
                # native TensorTensorReduce crashes TRN2; use TensorScalarPtr
                # (scalar_tensor_tensor) whose accum_out sums the result, then
                # chain partials into `accum` with per-partition adds.
                junk = wpool.tile([128, 6], F32, tag="junk")
                fresh = wpool.tile([128, 3], F32, tag="fresh")
                nc.vector.scalar_tensor_tensor(
                    out=junk[:, 0:6],
                    in0=d6[:, 0:6],
                    scalar=0.0,
                    in1=zeros6[:, 0:6],
                    op0=OP.max,
                    op1=OP.bypass,
                    accum_out=fresh[:, 0:1],
                )
                nc.vector.scalar_tensor_tensor(
                    out=junk[:, 0:4],
                    in0=sq[:, 0:4],
                    scalar=0.0,
                    in1=sq[:, 4:8],
                    op0=OP.add,
                    op1=OP.is_lt,
                    accum_out=fresh[:, 1:2],
                )
                nc.vector.scalar_tensor_tensor(
                    out=junk[:, 0:2],
                    in0=sq[:, 2:4],
                    scalar=0.0,
                    in1=sq[:, 4:6],
                    op0=OP.add,
                    op1=OP.is_lt,
                    accum_out=fresh[:, 2:3],
                )
                nc.vector.tensor_scalar_add(
                    out=accum[:, 0:1], in0=accum[:, 0:1], scalar1=fresh[:, 0:1]
                )
                nc.vector.tensor_scalar_add(
                    out=accum[:, 1:2], in0=accum[:, 1:2], scalar1=fresh[:, 1:2]
                )
                nc.vector.tensor_scalar_add(
                    out=accum[:, 1:2], in0=accum[:, 1:2], scalar1=fresh[:, 2:3]
                )

            nc.sync.dma_start(out=out_d[:, :], in_=accum[:, :])

    nc.compile()
    return nc


def _host_prep(modal1, modal2, targets):
    """Sort/rotate/shard the inputs; returns the 8 per-core input dicts."""
    m1 = np.ascontiguousarray(np.asarray(modal1, dtype=np.float32))
    m2 = np.ascontiguousarray(np.asarray(modal2, dtype=np.float32))
    t = np.asarray(targets).astype(np.int64).ravel()
    assert m1.shape == (N, D) and m2.shape == (N, D) and t.shape == (N,)

    order = np.argsort(t, kind="stable")
    ts = t[order]
    m1s = m1[order]
    m2s = m2[order]
    sq1 = np.einsum("nd,nd->n", m1s, m1s, dtype=np.float32).astype(np.float32)
    sq2 = np.einsum("nd,nd->n", m2s, m2s, dtype=np.float32).astype(np.float32)

    change = np.r_[True, ts[1:] != ts[:-1]]
    grp_start = np.where(change)[0]
    gidx = np.cumsum(change) - 1
    starts = grp_start[gidx]                      # class start per sorted row
    grp_end = np.r_[grp_start[1:], N]
    ends = grp_end[gidx]                          # class end per sorted row
    max_cls = int((grp_end - grp_start).max())
    assert max_cls <= ROT_MARGIN, f"class size {max_cls} exceeds rotation margin"

    in_maps = []
    for c in range(NCORES):
        rot = (RPC * c - ROT_MARGIN) % N
        cols = (rot + np.arange(N)) % N
        rows = np.arange(RPC * c, RPC * (c + 1))

        cs_loc = (starts[rows] - rot) % N
        ce_loc = (ends[rows] - rot) % N
        assert (cs_loc >= 1).all() and (ce_loc <= RPC + 2 * ROT_MARGIN).all()
        assert (ce_loc > cs_loc).all()
        blk = rows % RPC // 128
        ps_w = (cs_loc - 128 * blk).astype(np.float32)
        pe_w = (ce_loc - 128 * blk).astype(np.float32)
        assert (ps_w >= 0).all() and (pe_w <= WIN).all()

        pack = np.zeros((128, 24), np.float32)
        for b in range(NBLK):
            sl = slice(128 * b, 128 * (b + 1))
            pack[:, 0 + b] = ps_w[sl]
            pack[:, 4 + b] = pe_w[sl]
            pack[:, 8 + b] = ce_loc[sl].astype(np.float32)   # neg mask start
            pack[:, 12 + b] = cs_loc[sl].astype(np.float32)  # neg mask end
            pack[:, 16 + b] = sq1[rows][sl]
            pack[:, 20 + b] = sq2[rows][sl]

        in_maps.append(
            {
                "rhs1": np.ascontiguousarray(m1s[cols].T),
                "rhs2": np.ascontiguousarray(m2s[cols].T),
                "sqb1": np.ascontiguousarray(-sq1[cols][None, :]),
                "sqb2": np.ascontiguousarray(-sq2[cols][None, :]),
                "lhs1": np.ascontiguousarray((2.0 * m1s[rows]).T),
                "lhs2": np.ascontiguousarray((2.0 * m2s[rows]).T),
                "pack": pack,
                "ones": np.ones((1, 128), np.float32),
            }
        )
    return in_maps


_NC_CACHE = {}


def _get_nc():
    if "nc" not in _NC_CACHE:
        _NC_CACHE["nc"] = _build_program()
    return _NC_CACHE["nc"]


def kernel(modal1_inputs, modal2_inputs, targets, _trace=False):
    in_maps = _host_prep(modal1_inputs, modal2_inputs, targets)
    nc = _get_nc()
    res = run_bass_kernel_spmd(
        nc, in_maps, core_ids=list(range(NCORES)), trace=_trace
    )
    loss_sum = 0.0
    prec_sum = 0.0
    for r in res.results:
        loss_sum += float(r["out"][:, 0].sum(dtype=np.float64))
        prec_sum += float(r["out"][:, 1].sum(dtype=np.float64))
    denom = 6.0 * N
    out = (np.float32(loss_sum / denom), np.float32(prec_sum / denom))
    if _trace:
        return out, res
    return out
